# revision 16
# baseline (speedup 1.0000x reference)
"""CondAttnBlock Trainium2 kernel: GN -> 1x1conv q / linear k,v -> attention -> proj -> residual.

Sharding: data-parallel over batch B=32 across 8 NeuronCores (4 batches/core),
weights replicated, no collectives.

Key tricks:
  * fp32r matmuls (full-rate fp32 on the PE for free-dim >= 256).
  * q-projection eliminated via associativity: S = h^T (wq^T k^T); R = wq @ kT is
    4x smaller than q. GroupNorm folds into a per-channel affine absorbed into R
    (row scaling) plus a rank-1 row correction t[m] added with K=1 matmuls.
  * All biases applied as K=1 matmuls into the PSUM accumulation groups.
  * Softmax without max-subtraction (scores bounded), exp row-sums via ScalarE
    accum_out, P normalized per-partition, PE-transposed for the PV matmul.
  * rsqrt for GN via Newton iteration on VectorE (no ACT table-set switches).
"""

import sys

if "/opt/trn_rl_repo" not in sys.path:
    sys.path.insert(0, "/opt/trn_rl_repo")

from contextlib import ExitStack

import numpy as np

import concourse.bacc as bacc
import concourse.bass as bass
import concourse.mybir as mybir
import concourse.tile as tile

F32 = mybir.dt.float32
F32R = mybir.dt.float32r
I32 = mybir.dt.int32
AF = mybir.ActivationFunctionType
ALU = mybir.AluOpType
AX = mybir.AxisListType

B, C, S, M, D = 32, 512, 1024, 256, 768
G, CPG = 32, 16
NCORES = 8
BPC = B // NCORES  # batches per core
NCH = C // 128  # 4
NDH = D // 128  # 6
NMH = M // 128  # 2
NSH = S // 128  # 8
EPS = 1e-5
ATT_SCALE = float(C) ** -0.5
NELEM = float(CPG * S)  # elements per group
MAGIC = 0x5F3759DF


def r(ap):
    return ap.bitcast(F32R)


def dma_chunked(nc, dst_tile, src_2d, n, rnd=False):
    """DMA [n*128, F] HBM -> [128, n*F] SBUF tile (chunk i at cols [i*F, (i+1)*F))."""
    dst = dst_tile[:].rearrange("p (n f) -> p n f", n=n)
    src = src_2d.rearrange("(n p) f -> p n f", p=128)
    if rnd:
        dst, src = dst.bitcast(F32R), src.bitcast(F32R)
    nc.sync.dma_start(dst, src)


def build_program():
    nc = bacc.Bacc("TRN2", target_bir_lowering=False, debug=False)

    x_d = nc.dram_tensor("x", [BPC, C, S], F32, kind="ExternalInput").ap()
    y_d = nc.dram_tensor("y", [BPC, M, D], F32, kind="ExternalInput").ap()
    wq_d = nc.dram_tensor("wq", [C, C], F32, kind="ExternalInput").ap()
    wk_d = nc.dram_tensor("wk", [C, D], F32, kind="ExternalInput").ap()
    wv_d = nc.dram_tensor("wv", [C, D], F32, kind="ExternalInput").ap()
    wp_d = nc.dram_tensor("wp", [C, C], F32, kind="ExternalInput").ap()
    bq_d = nc.dram_tensor("bq", [C], F32, kind="ExternalInput").ap()
    bk_d = nc.dram_tensor("bk", [C], F32, kind="ExternalInput").ap()
    bv_d = nc.dram_tensor("bv", [C], F32, kind="ExternalInput").ap()
    bp_d = nc.dram_tensor("bp", [C], F32, kind="ExternalInput").ap()
    gns_d = nc.dram_tensor("gn_scale", [C], F32, kind="ExternalInput").ap()
    gnb_d = nc.dram_tensor("gn_bias", [C], F32, kind="ExternalInput").ap()
    eye_d = nc.dram_tensor("eye", [128, 128], F32, kind="ExternalInput").ap()
    ones_d = nc.dram_tensor("ones", [1, S], F32, kind="ExternalInput").ap()
    gmap_d = nc.dram_tensor("gmap", [C, G], F32, kind="ExternalInput").ap()
    gmapT_d = nc.dram_tensor("gmapT", [G, C], F32, kind="ExternalInput").ap()
    out_d = nc.dram_tensor("out", [BPC, C, S], F32, kind="ExternalOutput").ap()

    with tile.TileContext(nc) as tc, ExitStack() as ctx:
        wpool = ctx.enter_context(tc.tile_pool(name="w", bufs=1))
        xpool = ctx.enter_context(tc.tile_pool(name="x", bufs=2))
        ypool = ctx.enter_context(tc.tile_pool(name="y", bufs=2))
        kpool = ctx.enter_context(tc.tile_pool(name="kv", bufs=1))
        apool = ctx.enter_context(tc.tile_pool(name="att", bufs=1))
        ppool = ctx.enter_context(tc.tile_pool(name="pn", bufs=3))
        spool = ctx.enter_context(tc.tile_pool(name="st", bufs=2))
        scpool = ctx.enter_context(tc.tile_pool(name="scr", bufs=1))
        opool = ctx.enter_context(tc.tile_pool(name="o", bufs=3))
        pspool = ctx.enter_context(tc.tile_pool(name="ps", bufs=6, space="PSUM"))

        # ---------------- persistent constants / weights ----------------
        eye_sb = wpool.tile([128, 128], F32, tag="eye")
        nc.sync.dma_start(eye_sb[:], eye_d[:])
        eye_r = wpool.tile([128, 128], F32, tag="eyer")
        nc.sync.dma_start(r(eye_r[:]), r(eye_d[:]))
        ones_sb = wpool.tile([1, S], F32, tag="ones")
        nc.sync.dma_start(r(ones_sb[:]), r(ones_d[:]))
        gmap_sb = wpool.tile([128, NCH * G], F32, tag="gmap")
        dma_chunked(nc, gmap_sb, gmap_d, NCH)
        gmapT_sb = wpool.tile([G, C], F32, tag="gmapT")
        nc.sync.dma_start(gmapT_sb[:], gmapT_d[:])

        # wq stays natural: chunk ci of rows (c) at cols [ci*C, (ci+1)*C)
        wq_sb = wpool.tile([128, NCH * C], F32, tag="wq")
        dma_chunked(nc, wq_sb, wq_d, NCH, rnd=True)

        # bias rows [1, C]
        bk_row = wpool.tile([1, C], F32, tag="bk")
        nc.sync.dma_start(r(bk_row[:]), r(bk_d.rearrange("(a c) -> a c", a=1)))
        bv_row = wpool.tile([1, C], F32, tag="bv")
        nc.sync.dma_start(r(bv_row[:]), r(bv_d.rearrange("(a c) -> a c", a=1)))
        bp_row = wpool.tile([1, C], F32, tag="bp")
        nc.sync.dma_start(r(bp_row[:]), r(bp_d.rearrange("(a c) -> a c", a=1)))

        # per-channel columns [128, NCH] (col ci = channels ci*128..)
        gns_col = wpool.tile([128, NCH], F32, tag="gns")
        nc.sync.dma_start(gns_col[:], gns_d.rearrange("(n p) -> p n", p=128))
        gnb_col = wpool.tile([128, NCH], F32, tag="gnb")
        nc.sync.dma_start(gnb_col[:], gnb_d.rearrange("(n p) -> p n", p=128))
        bq_col = wpool.tile([128, NCH], F32, tag="bqc")
        nc.sync.dma_start(r(bq_col[:]), r(bq_d.rearrange("(n p) -> p n", p=128)))

        # ---------------- transpose wk, wv, wp on the PE ----------------
        # wkT/wvT: [768, 512] as 6 chunks [128(d), 512(c)]; wpT: [512, 512] 4 chunks [128(c), 512(o)]
        wkT = wpool.tile([128, NDH * C], F32, tag="wkT")
        wvT = wpool.tile([128, NDH * C], F32, tag="wvT")
        wpT = wpool.tile([128, NCH * C], F32, tag="wpT")
        with tc.tile_pool(name="wnat", bufs=1) as wnat:
            wk_nat = wnat.tile([128, NCH * D], F32, tag="wk_nat")
            dma_chunked(nc, wk_nat, wk_d, NCH)
            wv_nat = wnat.tile([128, NCH * D], F32, tag="wv_nat")
            dma_chunked(nc, wv_nat, wv_d, NCH)
            wp_nat = wnat.tile([128, NCH * C], F32, tag="wp_nat")
            dma_chunked(nc, wp_nat, wp_d, NCH)

            for w_nat, wT in ((wk_nat, wkT), (wv_nat, wvT)):
                for di in range(NDH):
                    pt = pspool.tile([128, C], F32, tag="ps")
                    for cj in range(NCH):
                        nc.tensor.matmul(
                            pt[:, cj * 128 : (cj + 1) * 128],
                            lhsT=w_nat[:, cj * D + di * 128 : cj * D + (di + 1) * 128],
                            rhs=eye_sb[:],
                            is_transpose=True,
                            start=(cj == 0),
                            stop=(cj == NCH - 1),
                        )
                    nc.scalar.copy(r(wT[:, di * C : (di + 1) * C]), pt[:])
            for ci in range(NCH):
                pt = pspool.tile([128, C], F32, tag="ps")
                for oj in range(NCH):
                    nc.tensor.matmul(
                        pt[:, oj * 128 : (oj + 1) * 128],
                        lhsT=wp_nat[:, oj * C + ci * 128 : oj * C + (ci + 1) * 128],
                        rhs=eye_sb[:],
                        is_transpose=True,
                        start=(oj == 0),
                        stop=(oj == NCH - 1),
                    )
                nc.scalar.copy(r(wpT[:, ci * C : (ci + 1) * C]), pt[:])

        # ---------------- batch loop ----------------
        for b in range(BPC):
            xb = xpool.tile([128, NCH * S], F32, tag="xb")
            dma_chunked(nc, xb, x_d[b], NCH, rnd=True)
            yb = ypool.tile([128, NMH * D], F32, tag="yb")
            dma_chunked(nc, yb, y_d[b], NMH)

            # y^T [768, 256]: 6 chunks [128(d), 256(m)]
            yT = ypool.tile([128, NDH * M], F32, tag="yT")
            for di in range(NDH):
                pt = pspool.tile([128, M], F32, tag="ps")
                for mj in range(NMH):
                    nc.tensor.matmul(
                        pt[:, mj * 128 : (mj + 1) * 128],
                        lhsT=yb[:, mj * D + di * 128 : mj * D + (di + 1) * 128],
                        rhs=eye_sb[:],
                        is_transpose=True,
                        start=(mj == 0),
                        stop=(mj == NMH - 1),
                    )
                nc.scalar.copy(r(yT[:, di * M : (di + 1) * M]), pt[:])

            # ---- GroupNorm statistics ----
            # per-channel sum (gpsimd) and sum of squares (DVE fused square+reduce)
            stat2 = spool.tile([128, 2 * NCH], F32, tag="stat2")
            for ci in range(NCH):
                nc.vector.reduce_sum(
                    stat2[:, 2 * ci : 2 * ci + 1], xb[:, ci * S : (ci + 1) * S], axis=AX.X
                )
                sq = scpool.tile([128, S], F32, tag="sq")
                nc.scalar.activation(
                    sq[:],
                    xb[:, ci * S : (ci + 1) * S],
                    AF.Square,
                    bias=0.0,
                    scale=1.0,
                    accum_out=stat2[:, 2 * ci + 1 : 2 * ci + 2],
                )
            # group sums via PE: [32, 2] = sum over channels in group
            gps = pspool.tile([G, 2], F32, tag="ps")
            for ci in range(NCH):
                nc.tensor.matmul(
                    gps[:],
                    lhsT=gmap_sb[:, ci * G : (ci + 1) * G],
                    rhs=stat2[:, 2 * ci : 2 * ci + 2],
                    start=(ci == 0),
                    stop=(ci == NCH - 1),
                )
            gstat = spool.tile([G, 2], F32, tag="gstat")  # [mean, E[x^2]]
            nc.vector.tensor_scalar_mul(gstat[:], gps[:], 1.0 / NELEM)
            msq = spool.tile([G, 1], F32, tag="msq")
            nc.vector.tensor_mul(msq[:], gstat[:, 0:1], gstat[:, 0:1])
            veps = spool.tile([G, 1], F32, tag="veps")  # var + eps
            nc.vector.scalar_tensor_tensor(
                veps[:], in0=msq[:], scalar=-1.0, in1=gstat[:, 1:2], op0=ALU.mult, op1=ALU.add
            )
            nc.vector.tensor_scalar_add(veps[:], veps[:], EPS)
            # rstd = rsqrt(veps) via Newton (bit-trick seed + 3 iterations)
            yk = spool.tile([G, 1], F32, tag="yk")
            nc.vector.tensor_scalar(
                yk[:].bitcast(I32), veps[:].bitcast(I32), 1, None, op0=ALU.logical_shift_right
            )
            nc.vector.tensor_scalar(
                yk[:].bitcast(I32), yk[:].bitcast(I32), MAGIC + 1, None, op0=ALU.subtract
            )
            nc.vector.tensor_scalar(
                yk[:].bitcast(I32), yk[:].bitcast(I32), -1, None, op0=ALU.bitwise_xor
            )
            for _ in range(3):
                y2 = spool.tile([G, 1], F32, tag="y2")
                nc.vector.tensor_mul(y2[:], yk[:], yk[:])
                nc.vector.tensor_mul(y2[:], y2[:], veps[:])
                nc.vector.tensor_scalar(y2[:], y2[:], -0.5, 1.5, op0=ALU.mult, op1=ALU.add)
                nc.vector.tensor_mul(yk[:], yk[:], y2[:])
            # bstat [32, 2] = (mean, rstd)
            bstat = spool.tile([G, 2], F32, tag="bstat")
            nc.vector.tensor_copy(bstat[:, 0:1], gstat[:, 0:1])
            nc.vector.tensor_copy(bstat[:, 1:2], yk[:])
            # broadcast back to channels: chan [128, 2*NCH] cols (mean_c, rstd_c)
            chan = spool.tile([128, 2 * NCH], F32, tag="chan")
            for ci in range(NCH):
                cps = pspool.tile([128, 2], F32, tag="ps")
                nc.tensor.matmul(
                    cps[:],
                    lhsT=gmapT_sb[:, ci * 128 : (ci + 1) * 128],
                    rhs=bstat[:],
                    start=True,
                    stop=True,
                )
                nc.scalar.copy(chan[:, 2 * ci : 2 * ci + 2], cps[:])
            # a = rstd * gn_scale ; e = gn_bias / a - mean   (so that e * a = d)
            a_col = spool.tile([128, NCH], F32, tag="acol")
            nc.vector.tensor_mul(a_col[:], chan[:, 1 : 2 * NCH : 2], gns_col[:])
            ra_col = spool.tile([128, NCH], F32, tag="racol")
            nc.vector.reciprocal(ra_col[:], a_col[:])
            etmp = spool.tile([128, NCH], F32, tag="etmp")
            nc.vector.tensor_mul(etmp[:], gnb_col[:], ra_col[:])
            e_col = spool.tile([128, NCH], F32, tag="ecol")
            nc.vector.tensor_sub(r(e_col[:]), etmp[:], chan[:, 0 : 2 * NCH : 2])

            # ---- k^T [512, 256]: chunks [128(c), 256(m)] ----
            kT = kpool.tile([128, NCH * M], F32, tag="kT")
            for ci in range(NCH):
                ps = pspool.tile([128, M], F32, tag="ps")
                for di in range(NDH):
                    nc.tensor.matmul(
                        ps[:],
                        lhsT=r(wkT[:, di * C + ci * 128 : di * C + (ci + 1) * 128]),
                        rhs=r(yT[:, di * M : (di + 1) * M]),
                        start=(di == 0),
                        stop=False,
                    )
                nc.tensor.matmul(
                    ps[:],
                    lhsT=r(bk_row[:, ci * 128 : (ci + 1) * 128]),
                    rhs=r(ones_sb[:, 0:M]),
                    start=False,
                    stop=True,
                )
                nc.scalar.copy(r(kT[:, ci * M : (ci + 1) * M]), ps[:])

            # ---- v [256, 512]: chunks [128(m), 512(c)] ----
            v_sb = kpool.tile([128, NMH * C], F32, tag="v")
            for mj in range(NMH):
                ps = pspool.tile([128, C], F32, tag="ps")
                for di in range(NDH):
                    nc.tensor.matmul(
                        ps[:],
                        lhsT=r(yT[:, di * M + mj * 128 : di * M + mj * 128 + 128]),
                        rhs=r(wvT[:, di * C : (di + 1) * C]),
                        start=(di == 0),
                        stop=False,
                    )
                nc.tensor.matmul(
                    ps[:],
                    lhsT=r(ones_sb[:, 0:128]),
                    rhs=r(bv_row[:]),
                    start=False,
                    stop=True,
                )
                nc.scalar.copy(r(v_sb[:, mj * C : (mj + 1) * C]), ps[:])

            # ---- Ra = diag(a) @ (wq @ kT)  [512, 256] chunks [128(c'), 256(m)] ----
            Ra = kpool.tile([128, NCH * M], F32, tag="Ra")
            for cj in range(NCH):
                ps = pspool.tile([128, M], F32, tag="ps")
                for ci in range(NCH):
                    nc.tensor.matmul(
                        ps[:],
                        lhsT=r(wq_sb[:, ci * C + cj * 128 : ci * C + cj * 128 + 128]),
                        rhs=r(kT[:, ci * M : (ci + 1) * M]),
                        start=(ci == 0),
                        stop=(ci == NCH - 1),
                    )
                nc.vector.tensor_scalar_mul(
                    r(Ra[:, cj * M : (cj + 1) * M]), ps[:], a_col[:, cj : cj + 1]
                )

            # ---- t row [1, 256] = e^T Ra + bq^T kT ----
            tps = pspool.tile([1, M], F32, tag="ps")
            for cj in range(NCH):
                nc.tensor.matmul(
                    tps[:],
                    lhsT=r(e_col[:, cj : cj + 1]),
                    rhs=r(Ra[:, cj * M : (cj + 1) * M]),
                    start=(cj == 0),
                    stop=False,
                )
            for ci in range(NCH):
                nc.tensor.matmul(
                    tps[:],
                    lhsT=r(bq_col[:, ci : ci + 1]),
                    rhs=r(kT[:, ci * M : (ci + 1) * M]),
                    start=False,
                    stop=(ci == NCH - 1),
                )
            t_row = spool.tile([1, M], F32, tag="trow")
            nc.scalar.copy(r(t_row[:]), tps[:])

            # ---- attention + projection, per s-half of 512 ----
            PT_sb = apool.tile([128, NMH * S], F32, tag="PT")  # [128(m), 2*1024(s)]
            for sh in range(2):
                # scores, softmax, transpose for the 4 s-chunks in this half
                for sp in range(2):  # pairs of s-chunks
                    pn_pair = []
                    for q in range(2):
                        sj = sh * 4 + sp * 2 + q
                        sps = pspool.tile([128, M], F32, tag="ps")
                        for cj in range(NCH):
                            nc.tensor.matmul(
                                sps[:],
                                lhsT=r(xb[:, cj * S + sj * 128 : cj * S + sj * 128 + 128]),
                                rhs=r(Ra[:, cj * M : (cj + 1) * M]),
                                start=(cj == 0),
                                stop=False,
                            )
                        nc.tensor.matmul(
                            sps[:],
                            lhsT=r(ones_sb[:, sj * 128 : (sj + 1) * 128]),
                            rhs=r(t_row[:]),
                            start=False,
                            stop=True,
                        )
                        P = ppool.tile([128, M], F32, tag="P")
                        rs = spool.tile([128, 1], F32, tag="rs")
                        nc.scalar.activation(
                            P[:], sps[:], AF.Exp, bias=0.0, scale=ATT_SCALE, accum_out=rs[:]
                        )
                        rinv = spool.tile([128, 1], F32, tag="rinv")
                        nc.vector.reciprocal(rinv[:], rs[:])
                        Pn = ppool.tile([128, M], F32, tag="Pn")
                        nc.vector.tensor_scalar_mul(r(Pn[:]), P[:], rinv[:])
                        pn_pair.append(Pn)
                    # transpose the pair: PT[m, s] for both m-chunks
                    for mj in range(NMH):
                        pt = pspool.tile([128, 256], F32, tag="ps")
                        for q in range(2):
                            nc.tensor.matmul(
                                r(pt[:, q * 128 : (q + 1) * 128]),
                                lhsT=r(pn_pair[q][:, mj * 128 : (mj + 1) * 128]),
                                rhs=r(eye_r[:]),
                                is_transpose=True,
                                start=(q == 0),
                                stop=(q == 1),
                            )
                        sj0 = sh * 4 + sp * 2
                        nc.scalar.copy(
                            r(PT_sb[:, mj * S + sj0 * 128 : mj * S + (sj0 + 2) * 128]), r(pt[:])
                        )

                # h_att^T [512, 512-half]: chunks [128(c), 512(s)]
                hatt = apool.tile([128, NCH * 512], F32, tag="hatt")
                for ci in range(NCH):
                    hps = pspool.tile([128, 512], F32, tag="ps")
                    for mj in range(NMH):
                        nc.tensor.matmul(
                            hps[:],
                            lhsT=r(v_sb[:, mj * C + ci * 128 : mj * C + ci * 128 + 128]),
                            rhs=r(PT_sb[:, mj * S + sh * 512 : mj * S + (sh + 1) * 512]),
                            start=(mj == 0),
                            stop=(mj == NMH - 1),
                        )
                    nc.vector.tensor_copy(r(hatt[:, ci * 512 : (ci + 1) * 512]), hps[:])

                # out^T chunks [128(o), 512(s)] = wpT^T hatt + bp + x
                for oj in range(NCH):
                    ops_ = pspool.tile([128, 512], F32, tag="ps")
                    for ci in range(NCH):
                        nc.tensor.matmul(
                            ops_[:],
                            lhsT=r(wpT[:, ci * C + oj * 128 : ci * C + oj * 128 + 128]),
                            rhs=r(hatt[:, ci * 512 : (ci + 1) * 512]),
                            start=(ci == 0),
                            stop=False,
                        )
                    nc.tensor.matmul(
                        ops_[:],
                        lhsT=r(bp_row[:, oj * 128 : (oj + 1) * 128]),
                        rhs=r(ones_sb[:, 0:512]),
                        start=False,
                        stop=True,
                    )
                    ot = opool.tile([128, 512], F32, tag="ot")
                    nc.vector.tensor_add(
                        ot[:], ops_[:], xb[:, oj * S + sh * 512 : oj * S + (sh + 1) * 512]
                    )
                    nc.sync.dma_start(
                        out_d[b, oj * 128 : (oj + 1) * 128, sh * 512 : (sh + 1) * 512], ot[:]
                    )

    nc.compile()
    return nc


def make_const_inputs():
    gmap = np.zeros((C, G), np.float32)
    gmap[np.arange(C), np.arange(C) // CPG] = 1.0
    return {
        "eye": np.eye(128, dtype=np.float32),
        "ones": np.ones((1, S), np.float32),
        "gmap": gmap,
        "gmapT": np.ascontiguousarray(gmap.T),
    }


_CACHE = {}


def kernel(_trace=False, **inputs):
    if "nc" not in _CACHE:
        _CACHE["nc"] = build_program()
    nc = _CACHE["nc"]

    x = np.ascontiguousarray(inputs["x"], np.float32).reshape(B, C, S)
    y = np.ascontiguousarray(inputs["y"], np.float32)
    shared = {
        k: np.ascontiguousarray(inputs[k], np.float32)
        for k in ("wq", "wk", "wv", "wp", "bq", "bk", "bv", "bp", "gn_scale", "gn_bias")
    }
    shared.update(make_const_inputs())

    in_maps = []
    for i in range(NCORES):
        m = dict(shared)
        m["x"] = np.ascontiguousarray(x[i * BPC : (i + 1) * BPC])
        m["y"] = np.ascontiguousarray(y[i * BPC : (i + 1) * BPC])
        in_maps.append(m)

    from concourse.bass_utils import run_bass_kernel_spmd

    res = run_bass_kernel_spmd(nc, in_maps, list(range(NCORES)), trace=_trace)
    _CACHE["exec_time_ns"] = res.exec_time_ns
    _CACHE["result"] = res
    out = np.concatenate([res.results[i]["out"] for i in range(NCORES)], axis=0)
    return out.reshape(B, C, 32, 32)
